# revision 11
# baseline (speedup 1.0000x reference)
"""Trainium2 Bass kernel for CRF Viterbi decode (nn_CRF_31585189495099).

Full inputs -> full outputs. Internally: shard batch B=64 across 8 NeuronCores
(8 sequences/core). Per core:
  Phase A: logits^T = (x @ kernel + bias)^T via PE fp32 matmul, laid out
           [b_local*U + u (partitions), t (free)] in SBUF, two 128-partition
           groups of 4 batches each.
  Phase B: Viterbi forward scan over t (serial, DVE): 3 ops/step/group:
           scores = trans + alpha[t-1]  (tensor_scalar, per-partition scalar)
           m      = max over u_prev      (tensor_reduce apply_transpose)
           alpha  = m + logit[t]         (tensor_scalar)
           fp32 op order matches the jax reference bit-for-bit given logits.
  Phase C: backpointers recomputed batched: scores -> 32x32 block transpose ->
           is_equal vs m -> * (31-u) -> reduce_max = 31-argmax (first-index
           exact, ties included).
Host: trivial O(B*T) backtrace over shipped bp + alpha (argmax/backtrace use
device-computed fp32 values only, so decisions match the device bit-exactly).
"""

import numpy as np

B, T, D, U = 64, 2048, 512, 32
NCORES = 8
BPC = B // NCORES            # batches per core
GROUPS = 2                   # 4 batches * 32 states = 128 partitions per group
BPG = 4                      # batches per group
F32 = None                   # set lazily (mybir import inside functions)

_cache = {}


def _build(t_len):
    import concourse.mybir as mybir
    from concourse import bacc
    import concourse.tile as tile

    f32 = mybir.dt.float32
    nc = bacc.Bacc("TRN2", target_bir_lowering=False, debug=False,
                   num_devices=NCORES)

    x_d = nc.dram_tensor("x", [BPC, t_len, D], f32, kind="ExternalInput")
    ker_d = nc.dram_tensor("ker", [D, U], f32, kind="ExternalInput")
    tr_d = nc.dram_tensor("trrep", [128, U], f32, kind="ExternalInput")
    bias_d = nc.dram_tensor("biasr", [128, 1], f32, kind="ExternalInput")
    w31_d = nc.dram_tensor("w31", [128, U], f32, kind="ExternalInput")
    alpha_d = nc.dram_tensor("alpha_out", [GROUPS * 128, t_len], f32,
                             kind="ExternalOutput")
    bp_d = nc.dram_tensor("bp_out", [GROUPS * 128, t_len], f32,
                          kind="ExternalOutput")

    NT = t_len // 512 if t_len >= 512 else 1      # 512-wide time tiles
    TT = min(512, t_len)
    assert t_len % TT == 0

    with tile.TileContext(nc) as tc:
        with (
            tc.tile_pool(name="const", bufs=1) as cpool,
            tc.tile_pool(name="big", bufs=1) as bigpool,
            tc.tile_pool(name="xin", bufs=4) as xpool,
            tc.tile_pool(name="ps", bufs=1, space="PSUM") as pspool,
            tc.tile_pool(name="scr", bufs=1) as spool,
        ):
            ker_sb = cpool.tile([128, 4 * U], f32)      # 4 K-chunks as col blocks
            for kc in range(4):
                nc.sync.dma_start(out=ker_sb[:, kc * U:(kc + 1) * U],
                                  in_=ker_d[kc * 128:(kc + 1) * 128, :])
            tr_sb = cpool.tile([128, U], f32)
            nc.sync.dma_start(out=tr_sb[:, :], in_=tr_d[:, :])
            bias_sb = cpool.tile([128, 1], f32)
            nc.sync.dma_start(out=bias_sb[:, :], in_=bias_d[:, :])
            w31_sb = cpool.tile([128, U], f32)
            nc.sync.dma_start(out=w31_sb[:, :], in_=w31_d[:, :])

            logits = [bigpool.tile([128, t_len], f32, tag=f"lg{g}", name=f"lg{g}")
                      for g in range(GROUPS)]
            alpha = [bigpool.tile([128, t_len], f32, tag=f"al{g}", name=f"al{g}")
                     for g in range(GROUPS)]
            mst = [bigpool.tile([128, t_len], f32, tag=f"m{g}", name=f"m{g}")
                   for g in range(GROUPS)]

            # ---------------- Phase A: logits^T ----------------
            # x loaded in natural [t, d] layout (2 KB contiguous rows), then
            # PE-transposed 128x128 through PSUM into xT tiles for the matmul.
            from concourse.masks import make_identity
            ident = cpool.tile([128, 128], f32)
            make_identity(nc, ident[:, :])
            NS = TT // 128
            for tt in range(NT):
                for g in range(GROUPS):
                    t0 = tt * TT
                    ps_lg = pspool.tile([128, TT], f32, tag="pslg", bufs=2)
                    for bl in range(BPG):
                        b = g * BPG + bl
                        xTk = [xpool.tile([128, TT], f32, tag=f"xTk{kc}",
                                          name=f"xTk{kc}")
                               for kc in range(4)]
                        for ts4 in range(NS):
                            xa = xpool.tile([128, D], f32, tag="xa")
                            nc.sync.dma_start(
                                out=xa[:, :],
                                in_=x_d[b, t0 + ts4 * 128:t0 + (ts4 + 1) * 128, :])
                            for kc in range(4):
                                pt = pspool.tile([128, 128], f32, tag="pt",
                                                 bufs=4)
                                nc.tensor.transpose(
                                    pt[:, :], xa[:, kc * 128:(kc + 1) * 128],
                                    ident[:, :])
                                nc.scalar.copy(
                                    xTk[kc][:, ts4 * 128:(ts4 + 1) * 128],
                                    pt[:, :])
                        for kc in range(4):
                            nc.tensor.matmul(
                                ps_lg[bl * U:(bl + 1) * U, :],
                                ker_sb[:, kc * U:(kc + 1) * U],
                                xTk[kc][:, :],
                                start=(kc == 0), stop=(kc == 3),
                                tile_position=(0, bl * U))
                    # bias add + PSUM -> SBUF on ScalarE (keeps DVE free so the
                    # scan can start while later time-tiles are still in phase A)
                    nc.scalar.activation(logits[g][:, t0:t0 + TT], ps_lg[:, :],
                                         mybir.ActivationFunctionType.Identity,
                                         bias=bias_sb[:, 0:1])

            # ------- Phase B + C: forward scan with pipelined backpointers ----
            # Backpointer blocks are recomputed batched and staged through the
            # scan stream: gpsimd does the elementwise passes while the DVE
            # (scan-bound) only takes the 32x32 transpose + reduce per block.
            TC = 128
            for g in range(GROUPS):
                nc.vector.tensor_copy(alpha[g][:, 0:1], logits[g][:, 0:1])

            blocks = [(s, min(TC, t_len - s)) for s in range(1, t_len, TC)]
            blk_of = {}
            for i, (s, L) in enumerate(blocks):
                for t in range(s, s + L):
                    blk_of[t] = (i, s)
            with tc.tile_pool(name="bp", bufs=2) as bpool:
                sb_cur = {}   # g -> current scores-block tile

                def c_flush(g, i):
                    # argmax over u_prev of the captured scores block
                    s, L = blocks[i]
                    sb = sb_cur[g]
                    sbT = bpool.tile([128, TC * U], f32, tag="sbT",
                                     name=f"sbT{g}_{i}", bufs=2)
                    enc = bpool.tile([128, TC], f32, tag="enc",
                                     name=f"enc{g}_{i}", bufs=2)
                    nc.vector.transpose(sbT[:, :L * U], sb[:, :L * U])
                    sT3 = sbT[:, :L * U].rearrange("p (t u) -> p t u", u=U)
                    eq3 = sb[:, :L * U].rearrange("p (t u) -> p t u", u=U)
                    m_bc = mst[g][:, s:s + L][:, :, None] \
                        .broadcast_to([128, L, U])
                    nc.vector.tensor_tensor(eq3, sT3, m_bc,
                                            op=mybir.AluOpType.is_equal)
                    w_bc = w31_sb[:, None, :].broadcast_to([128, L, U])
                    nc.vector.tensor_tensor(sT3, eq3, w_bc,
                                            op=mybir.AluOpType.mult)
                    nc.vector.tensor_reduce(enc[:, :L], sT3,
                                            axis=mybir.AxisListType.X,
                                            op=mybir.AluOpType.max)
                    nc.sync.dma_start(out=bp_d[g * 128:(g + 1) * 128, s:s + L],
                                      in_=enc[:, :L])

                for t in range(1, t_len):
                    i, s = blk_of[t]
                    for g in range(GROUPS):
                        if t == s:  # new scores block
                            sb_cur[g] = bpool.tile([128, TC * U], f32,
                                                   tag="sb", bufs=3,
                                                   name=f"sb{g}_{i}")
                        sc = sb_cur[g][:, (t - s) * U:(t - s + 1) * U]
                        # scores captured straight into the block tile
                        nc.vector.tensor_scalar(sc, tr_sb[:, :],
                                                alpha[g][:, t - 1:t], None,
                                                op0=mybir.AluOpType.add)
                        nc.vector.tensor_reduce(mst[g][:, t:t + 1], sc,
                                                axis=mybir.AxisListType.X,
                                                op=mybir.AluOpType.max,
                                                apply_transpose=True)
                        nc.vector.tensor_scalar(alpha[g][:, t:t + 1],
                                                mst[g][:, t:t + 1],
                                                logits[g][:, t:t + 1], None,
                                                op0=mybir.AluOpType.add)
                    if t == blocks[i][0] + blocks[i][1] - 1:  # block complete
                        for g in range(GROUPS):
                            c_flush(g, i)
                for g in range(GROUPS):
                    nc.sync.dma_start(out=alpha_d[g * 128:(g + 1) * 128, :],
                                      in_=alpha[g][:, :])
    nc.compile()
    return nc


def _get_nc(t_len):
    if t_len not in _cache:
        _cache[t_len] = _build(t_len)
    return _cache[t_len]


def kernel(x, nwords, kernel, chain_kernel, bias):
    from concourse.bass_utils import run_bass_kernel_spmd

    x = np.ascontiguousarray(np.asarray(x, np.float32))
    ker = np.ascontiguousarray(np.asarray(kernel, np.float32))
    tr = np.asarray(chain_kernel, np.float32)
    bias = np.asarray(bias, np.float32)
    nwords = np.asarray(nwords, np.int32)

    t_len = x.shape[1]
    nc = _get_nc(t_len)

    tr_rep = np.ascontiguousarray(np.tile(tr, (4, 1)).astype(np.float32))
    bias_rep = np.ascontiguousarray(np.tile(bias, 4)[:, None].astype(np.float32))
    w31 = np.ascontiguousarray(
        np.broadcast_to((31 - np.arange(U)).astype(np.float32), (128, U)))

    xs = x.reshape(NCORES, BPC, t_len, D)
    in_maps = [{"x": np.ascontiguousarray(xs[c]), "ker": ker,
                "trrep": tr_rep, "biasr": bias_rep, "w31": w31}
               for c in range(NCORES)]
    res = run_bass_kernel_spmd(nc, in_maps, core_ids=list(range(NCORES)))

    pred = np.zeros((B, t_len), np.int32)
    score = np.zeros(B, np.float32)
    # rows: group g, batch-in-group bl, state u -> partition g*128+bl*32+u
    alpha_all = np.zeros((B, U, t_len), np.float32)
    bp_all = np.zeros((B, U, t_len), np.int32)
    for c in range(NCORES):
        a = res.results[c]["alpha_out"].reshape(GROUPS, BPG, U, t_len)
        e = res.results[c]["bp_out"].reshape(GROUPS, BPG, U, t_len)
        for g in range(GROUPS):
            for bl in range(BPG):
                b = c * BPC + g * BPG + bl
                alpha_all[b] = a[g, bl]
                bp_all[b] = (31.0 - e[g, bl]).astype(np.int32)

    L = np.maximum(nwords.astype(np.int64), 1)
    lastcol = alpha_all[np.arange(B), :, L - 1]          # [B, U]
    last_tag = lastcol.argmax(1).astype(np.int32)
    score = lastcol.max(1).astype(np.float32)

    tags = last_tag.copy()
    arange = np.arange(B)
    for t in range(t_len - 1, 0, -1):
        active = t <= (L - 1)
        nxt = bp_all[arange, tags, t]
        tags = np.where(active, nxt, tags)
        pred[:, t - 1] = tags
    pred[:, 0] = tags
    tmask = np.arange(t_len)[None, :] >= (L - 1)[:, None]
    pred = np.where(tmask, last_tag[:, None], pred).astype(np.int32)
    return pred, score
